# revision 19
# baseline (speedup 1.0000x reference)
"""GQA attention block (B=2, L=2048, D=4096, H=32, HKV=8, RoPE, causal) on 8
Trainium2 NeuronCores.

Sharding: core c -> batch b=c//4, head-group g=c%4 (8 Q heads + 2 KV heads per
core).  Each core computes x[b] @ wq_g/wk_g/wv_g projections, RoPE, causal
attention for its heads, and a partial output projection against its slice of
wo (row-sharded contraction).  The host sums the 4 partials per batch element
(the all-reduce of the tensor-parallel output projection, done at unshard).

All matmul operands are bfloat16 (PSUM accumulation stays fp32): bf16 enables
the PE fast-weight-load path (disabled for fp32) and halves HBM/SBUF traffic.
Device layouts put every matmul contraction on the partition axis; the host
pre-tiles x and all weights into the exact SBUF tile layouts so every DMA is
a single fully-contiguous read.  wq/wk rows are pair-permuted ([evens|odds]
per head) so RoPE becomes a partition half-swap, folded into partition-offset
operands of the sin multiply (no explicit swap copies).

Projections run in a single pass over the full D=4096 contraction (weights
held as [128, 32*128] tiles), with the two l-chunks of a pair sharing each
stationary tile back-to-back.  Scores are computed transposed,
S^T[j, l] = K^T.T @ Q^T, so softmax probs feed the PV matmul with no on-chip
transposes.  Attention processes head PAIRS so the kT/v stationary tiles are
shared by consecutive matmuls.  Diagonal score tiles are column-restricted
(queries >= key block only), trimming ~15% of score/exp/PV/den work; the
remaining triangular mask is a gpsimd affine_select zeroing exp output
(exp(s+m) == exp(s)*[m==0] exactly for the 0/-1e9 mask).  The softmax
denominator accumulates through an all-ones stationary matmul over the same
E^T tiles (partition-broadcast for free); normalization is
reciprocal_approx_fast + multiply.  Score matmuls are issued LOOKAHEAD tiles
ahead of the PV/denominator matmuls so the scalar-engine exp latency stays
off the PE critical path.

DMA queues are spread across engines (x on sync, wqkv on vector, wo + tables
on scalar, output on gpsimd) so loads for different phases don't serialize
behind each other.
"""

import numpy as np
import ml_dtypes

import concourse.mybir as mybir
import concourse.tile as tile
from concourse import bacc, bass_utils

B, L, D = 2, 2048, 4096
H, HKV, HD = 32, 8, 128
NCORES = 8
GROUPS = 4                # head groups (cores per batch element)
QH = H // GROUPS          # 8 q heads per core
KVH = HKV // GROUPS       # 2 kv heads per core
NM = QH + 2 * KVH         # 12 projection m-tiles per core (q0..7, k0..1, v0..1)
LC = 512                  # l-chunk (matmul moving free dim)
DT = D // 128             # 32 contraction tiles for projections
SCALE = 1.0 / float(np.sqrt(HD))
LOOKAHEAD = 3             # score-matmul jts in flight ahead of PV

f32 = mybir.dt.float32
bf16 = mybir.dt.bfloat16
BF16NP = ml_dtypes.bfloat16


def build_nc(seq_len=L):
    nlc = seq_len // LC

    lc_pairs = [
        [lc for lc in (2 * i, 2 * i + 1) if lc < nlc] for i in range((nlc + 1) // 2)
    ]
    max_plc = max(len(p) for p in lc_pairs)
    nc = bacc.Bacc(trn_type="TRN2")
    # host-pre-tiled operands: every DMA below is a contiguous read
    x_tl = nc.dram_tensor(
        "x_tl", [len(lc_pairs) * 8, 128, 4 * max_plc * LC], bf16, kind="ExternalInput"
    )
    wqkv_tl = nc.dram_tensor(
        "wqkv_tl", [NM * 2, 128, 16 * 128], bf16, kind="ExternalInput"
    )
    wo_tl = nc.dram_tensor("wo_tl", [D // 128, 128, QH * 128], bf16, kind="ExternalInput")
    cosT = nc.dram_tensor("cosT", [64, seq_len], f32, kind="ExternalInput")
    sinT = nc.dram_tensor("sinT", [64, seq_len], f32, kind="ExternalInput")
    ones128 = nc.dram_tensor("ones128", [128, 128], bf16, kind="ExternalInput")
    ident = nc.dram_tensor("ident", [128, 128], bf16, kind="ExternalInput")
    outT = nc.dram_tensor("outT", [D, seq_len], bf16, kind="ExternalOutput")

    with tile.TileContext(nc) as tc:
        with (
            tc.tile_pool(name="persist", bufs=1) as pp,
            tc.tile_pool(name="xp", bufs=1) as xp,
            tc.tile_pool(name="qp", bufs=1) as qp,
            tc.tile_pool(name="op", bufs=1) as op_,
            tc.tile_pool(name="wp", bufs=2) as wp,
            tc.tile_pool(name="ep", bufs=10) as ep,
            tc.tile_pool(name="tp", bufs=2) as tp,
            tc.tile_pool(name="outp", bufs=2) as outp,
            tc.tile_pool(name="mmps", bufs=4, space="PSUM") as mmps,
            tc.tile_pool(name="ops", bufs=2, space="PSUM") as ops_,
            tc.tile_pool(name="dps", bufs=2, space="PSUM") as dps,
        ):
            kT_t = {
                (kv, lc): pp.tile(
                    [128, LC], bf16, tag=f"kT_{kv}_{lc}", name=f"kT_{kv}_{lc}"
                )
                for kv in range(KVH) for lc in range(nlc)
            }
            v_t = {
                lc: pp.tile(
                    [128, 4, KVH * HD], bf16, tag=f"v_{lc}", name=f"v_{lc}"
                )
                for lc in range(nlc)
            }
            cs2 = pp.tile([128, seq_len], f32)
            sn2 = pp.tile([128, seq_len], f32)
            o128 = pp.tile([128, 128], bf16)
            idt = pp.tile([128, 128], bf16)

            nc.gpsimd.dma_start(cs2[0:64, :], cosT.ap())
            nc.gpsimd.dma_start(cs2[64:128, :], cosT.ap())
            nc.gpsimd.dma_start(sn2[0:64, :], sinT.ap())
            nc.gpsimd.dma_start(sn2[64:128, :], sinT.ap())
            # rotate-half form: out = q*cs2 + swap(q)*sn2 with sn2 = [-sin | sin]
            nc.vector.tensor_scalar_mul(sn2[0:64, :], sn2[0:64, :], -1.0)
            nc.gpsimd.dma_start(o128[:], ones128.ap())
            nc.gpsimd.dma_start(idt[:], ident.ap())

            for pi, lcs in enumerate(lc_pairs):
                plc = len(lcs)
                o2 = op_.tile([128, QH, plc * LC], bf16, tag="o2")
                q_pr = qp.tile([128, QH, plc * LC], bf16, tag="q")
                # ---- phase 1: projections, full-D contraction in one pass ----
                # x held as 8 separate tiles so the first matmuls only wait on
                # the first 1MB DMA, not the whole 8MB load.
                x_ts = []
                for c in range(8):
                    x_c = xp.tile(
                        [128, 4, plc * LC], bf16, tag=f"x{c}", name=f"x{c}"
                    )
                    nc.sync.dma_start(
                        x_c[:],
                        x_tl.ap()[pi * 8 + c]
                        .rearrange("p (a b) -> p a b", a=4)[:, :, : plc * LC],
                    )
                    x_ts.append(x_c)
                for mi in range(NM):
                    kind = "q" if mi < QH else ("k" if mi < QH + KVH else "v")
                    m = mi if mi < QH else (mi - QH if kind == "k" else mi - QH - KVH)
                    wt = wp.tile([128, 32 * 128], bf16, tag="w")
                    nc.scalar.dma_start(wt[:, : 16 * 128], wqkv_tl.ap()[mi * 2])
                    nc.scalar.dma_start(wt[:, 16 * 128:], wqkv_tl.ap()[mi * 2 + 1])
                    # both l-chunks' accumulators stay live so consecutive
                    # matmuls share each stationary tile (lci inner)
                    pss = [
                        mmps.tile([128, LC], f32, tag="mm", name=f"ps{_i}")
                        for _i in range(plc)
                    ]
                    for dt in range(DT):
                        for lci in range(plc):
                            nc.tensor.matmul(
                                pss[lci][:],
                                wt[:, dt * 128:(dt + 1) * 128],
                                x_ts[dt // 4][:, dt % 4, lci * LC:(lci + 1) * LC],
                                start=(dt == 0), stop=(dt == DT - 1),
                            )
                    for lci, lc in enumerate(lcs):
                        ps = pss[lci]
                        if kind in ("q", "k"):
                            lsl = slice(lc * LC, (lc + 1) * LC)
                            t1 = tp.tile([128, LC], f32, tag="t1")
                            nc.vector.tensor_mul(t1[:], ps[:], cs2[:, lsl])
                            t2 = tp.tile([128, LC], f32, tag="t2")
                            nc.vector.tensor_mul(
                                t2[0:64, :], ps[64:128, :], sn2[0:64, lsl]
                            )
                            nc.vector.tensor_mul(
                                t2[64:128, :], ps[0:64, :], sn2[64:128, lsl]
                            )
                            dst = (
                                q_pr[:, m, lci * LC:(lci + 1) * LC]
                                if kind == "q"
                                else kT_t[(m, lc)][:]
                            )
                            nc.vector.tensor_tensor(
                                dst, t1[:], t2[:], mybir.AluOpType.add
                            )
                        else:
                            vt = tp.tile([128, LC], bf16, tag="vt")
                            nc.vector.tensor_copy(vt[:], ps[:])
                            for jj in range(4):
                                pt = mmps.tile([128, 128], bf16, tag="mm")
                                nc.tensor.transpose(
                                    pt[:], vt[:, jj * 128:(jj + 1) * 128], idt[:]
                                )
                                nc.vector.tensor_copy(
                                    v_t[lc][:, jj, m * 128:(m + 1) * 128], pt[:]
                                )
                for lci, lc in enumerate(lcs):
                    # ---- phase 2: causal attention for queries in this l-chunk ----
                    njt = 4 * (lc + 1)
                    for hg in range(QH // 2):
                        kv = hg // 2
                        hs = [2 * hg, 2 * hg + 1]
                        po = {
                            h: ops_.tile([128, LC], f32, tag="po", name=f"po{h}")
                            for h in hs
                        }
                        pden = {
                            h: dps.tile([128, LC], f32, tag="pden", name=f"pden{h}")
                            for h in hs
                        }
                        e_tiles = {}

                        def emit_score(jt, hs=hs, kv=kv, e_tiles=e_tiles, lc=lc,
                                       lci=lci, q_pr=q_pr):
                            dg = jt - 4 * lc
                            qoff = max(dg, 0) * 128
                            w = LC - qoff
                            for h in hs:
                                psS = mmps.tile([128, w], f32, tag="mm", name=f"psS{h}_{jt}")
                                nc.tensor.matmul(
                                    psS[:],
                                    kT_t[(kv, jt // 4)][
                                        :, (jt % 4) * 128:(jt % 4 + 1) * 128
                                    ],
                                    q_pr[:, h, lci * LC + qoff:(lci + 1) * LC],
                                    start=True, stop=True,
                                )
                                e = ep.tile([128, w], bf16, tag="e", name=f"e{h}_{jt}")
                                nc.scalar.activation(
                                    e[:], psS[:], mybir.ActivationFunctionType.Exp,
                                    scale=SCALE,
                                )
                                if dg >= 0:
                                    nc.gpsimd.affine_select(
                                        out=e[:], in_=e[:],
                                        compare_op=mybir.AluOpType.is_ge,
                                        fill=0.0,
                                        base=0,
                                        pattern=[[1, w]],
                                        channel_multiplier=-1,
                                    )
                                e_tiles[(h, jt)] = e

                        for jt in range(min(LOOKAHEAD, njt)):
                            emit_score(jt)
                        for jt in range(njt):
                            if jt + LOOKAHEAD < njt:
                                emit_score(jt + LOOKAHEAD)
                            dg = jt - 4 * lc
                            qoff = max(dg, 0) * 128
                            vstat = v_t[jt // 4][:, jt % 4, kv * 128:(kv + 1) * 128]
                            for h in hs:
                                nc.tensor.matmul(
                                    po[h][:, qoff:],
                                    vstat,
                                    e_tiles[(h, jt)][:],
                                    start=(jt == 0), stop=(jt == njt - 1),
                                    skip_group_check=True,
                                )
                            for h in hs:
                                nc.tensor.matmul(
                                    pden[h][:, qoff:], o128[:],
                                    e_tiles.pop((h, jt))[:],
                                    start=(jt == 0), stop=(jt == njt - 1),
                                    skip_group_check=True,
                                )
                        for h in hs:
                            rec = tp.tile([128, LC], f32, tag="rec")
                            nc.vector.reciprocal_approx_fast(out=rec[:], in_=pden[h][:])
                            nc.vector.tensor_mul(
                                o2[:, h, lci * LC:(lci + 1) * LC], po[h][:], rec[:]
                            )
                # ---- phase 3: partial output projection for the pair ----
                for nt in range(D // 128):
                    wo_t = wp.tile([128, QH * 128], bf16, tag="wo")
                    nc.scalar.dma_start(wo_t[:], wo_tl.ap()[nt])
                    psos = [
                        mmps.tile([128, LC], f32, tag="mm", name=f"pso{_i}")
                        for _i in range(plc)
                    ]
                    for h in range(QH):
                        for lci in range(plc):
                            nc.tensor.matmul(
                                psos[lci][:], wo_t[:, h * 128:(h + 1) * 128],
                                o2[:, h, lci * LC:(lci + 1) * LC],
                                start=(h == 0), stop=(h == QH - 1),
                            )
                    for lci, lc in enumerate(lcs):
                        ob = outp.tile([128, LC], bf16, tag="ob")
                        nc.vector.tensor_copy(ob[:], psos[lci][:])
                        nc.gpsimd.dma_start(
                            outT.ap()[nt * 128:(nt + 1) * 128, lc * LC:(lc + 1) * LC],
                            ob[:],
                        )
    nc.compile()
    return nc


_PERM = np.concatenate([np.arange(0, HD, 2), np.arange(1, HD, 2)])


def _tile_weight(wT):
    """[D, M] (transposed weight) -> [M//128 * 2, 128, 16*128] contiguous tiles:
    tile (m, half)[p, dt8, mc] = wT[(half*16+dt8)*128 + p, m*128 + mc]."""
    Dd, M = wT.shape
    w = wT.reshape(2, 16, 128, M // 128, 128)         # [half, dt8, p, m, mc]
    w = w.transpose(3, 0, 2, 1, 4)                     # [m, half, p, dt8, mc]
    return np.ascontiguousarray(w.reshape(M // 128 * 2, 128, 16 * 128)).astype(BF16NP)


def shard_inputs(x, wq, wk, wv, wo, cos, sin, mask, seq_len=L):
    """Build the 8 per-core input maps (host pre-tiling)."""
    nlc = seq_len // LC
    cosT = np.ascontiguousarray(cos[:seq_len].T, dtype=np.float32)
    sinT = np.ascontiguousarray(sin[:seq_len].T, dtype=np.float32)
    ones128 = np.ones((128, 128), BF16NP)
    ident = np.eye(128, dtype=BF16NP)

    lc_pairs = [
        [lc for lc in (2 * i, 2 * i + 1) if lc < nlc] for i in range((nlc + 1) // 2)
    ]
    max_plc = max(len(p) for p in lc_pairs)
    x_tls = []
    for b in range(B):
        xT = x[b, :seq_len].T.astype(np.float32)       # [D, seq]
        xv = xT.reshape(8, 4, 128, seq_len)            # [hq(half*4+quar), dt4, p, l]
        x_tl = np.zeros((len(lc_pairs) * 8, 128, 4 * max_plc * LC), BF16NP)
        for pi, lcs in enumerate(lc_pairs):
            cols = np.concatenate([np.arange(lc * LC, (lc + 1) * LC) for lc in lcs])
            blk = xv[:, :, :, cols]                    # [hq, dt4, p, plc*LC]
            blk = blk.transpose(0, 2, 1, 3)            # [hq, p, dt4, plc*LC]
            x_tl[pi * 8:(pi + 1) * 8, :, : len(cols) * 4] = blk.reshape(8, 128, -1)
        x_tls.append(x_tl)

    def permute_rows(w):
        nh = w.shape[0] // HD
        wp_ = w.reshape(nh, HD, -1)[:, _PERM, :]
        return wp_.reshape(w.shape)

    in_maps = []
    for c in range(NCORES):
        b, g = divmod(c, GROUPS)
        wq_g = permute_rows(wq[QH * HD * g:QH * HD * (g + 1)])
        wk_g = permute_rows(wk[KVH * HD * g:KVH * HD * (g + 1)])
        wv_g = wv[KVH * HD * g:KVH * HD * (g + 1)]
        wo_g = wo[:, QH * HD * g:QH * HD * (g + 1)]
        wqkv_tl = np.concatenate(
            [_tile_weight(wq_g.T), _tile_weight(wk_g.T), _tile_weight(wv_g.T)], axis=0
        )
        woT = wo_g.T.astype(np.float32)                # [1024, D]
        wov = woT.reshape(QH, 128, D // 128, 128)      # [kt, p, nt, n]
        wov = wov.transpose(2, 1, 0, 3)                # [nt, p, kt, n]
        wo_tl = np.ascontiguousarray(
            wov.reshape(D // 128, 128, QH * 128)
        ).astype(BF16NP)
        in_maps.append({
            "x_tl": x_tls[b],
            "wqkv_tl": wqkv_tl,
            "wo_tl": wo_tl,
            "cosT": cosT,
            "sinT": sinT,
            "ones128": ones128,
            "ident": ident,
        })
    return in_maps


def gather_output(results, seq_len=L):
    out = np.zeros((B, seq_len, D), np.float32)
    for c in range(NCORES):
        b = c // GROUPS
        out[b] += results[c]["outT"].T.astype(np.float32)
    return out


_nc_cache = {}


def _get_nc(seq_len=L):
    if seq_len not in _nc_cache:
        _nc_cache[seq_len] = build_nc(seq_len)
    return _nc_cache[seq_len]


def run_sharded(inputs, trace=False, tmpdir=None):
    nc = _get_nc()
    in_maps = shard_inputs(**inputs)
    res = bass_utils.run_bass_kernel_spmd(
        nc, in_maps, core_ids=list(range(NCORES)), trace=trace, tmpdir=tmpdir
    )
    return gather_output(res.results), res


def kernel(**inputs) -> np.ndarray:
    out, _ = run_sharded(inputs)
    return out
